# revision 10
# baseline (speedup 1.0000x reference)
"""DockingScorePredictor Trainium2 kernel.

Data-parallel over complexes: 8 cores, one complex (512 protein x 64 ligand
atoms) per core.  Only pairs inside the 8A cutoff matter (~43%), so the host
compacts valid pairs into a dense stream of NT tiles x 512 pairs.

Per pair the layer-1 preactivation is
  z1 = W1a.T hp + W1b.T hl + b1 + W1c.T rb(d)
and hp/hl depend only on the atom *type* (20/16 types), so z1 collapses to a
single K=68 matmul against [rb(32); onehot_ptype(20); onehot_ltype(16)] with
a fused weight [W1c; prot_emb@W1a; lig_emb@W1b + b1].  Device work per tile:
3 matmuls (K=68 z1, W2, W3; all fp16, N=512) + 3 relu evacuations split
DVE/ACT (~35%/65% of relu3 to balance engine time), with the pair-sum fused
into the accum_out port.  Constants arrive in 2 blob DMAs and rhs in 4-tile
chunks to keep the Sync sequencer (~0.6us per dma_start) off the critical
path.

Pad columns (zero rhs) contribute relu(W3.T relu(b2) + b3) per column; that
constant is extracted on-device from a guaranteed-pad column and subtracted
as npad * cvec, so masking costs no matmul and no host-side rounding model.
"""
import numpy as np
from contextlib import ExitStack

import concourse.bass as bass
import concourse.bacc as bacc
import concourse.tile as tile
from concourse import mybir
from concourse import bass_utils

F32 = mybir.dt.float32
FP16 = mybir.dt.float16
AF = mybir.ActivationFunctionType
ALU = mybir.AluOpType

B, P, L = 8, 512, 64
H, RB = 128, 32
NPT, NLT = 20, 16
CUTOFF = 8.0
N_CORES = 8
K1 = RB + NPT + NLT          # 68: contraction dim of the fused z1 matmul
NT_DEFAULT = 31              # tiles of 512 pairs
CHUNK = 4                    # rhs tiles per dma_start
WIDTH = 0.5 * CUTOFF / RB + 1e-8
HBLOB = 4 * H + 1            # fp16 blob cols: W1z, W2, W3, Wr1, Wr2

_CACHE = {}
_DEBUG = False


def _build_nc(nt):
    nc = bacc.Bacc("TRN2", target_bir_lowering=False, debug=False,
                   num_devices=N_CORES)
    d = {}

    def inp(name, shape, dt):
        d[name] = nc.dram_tensor(name, shape, dt, kind="ExternalInput").ap()

    inp("rhsG", [K1, nt * 512], FP16)
    inp("wb", [H, HBLOB], FP16)   # cols: W1z(128) W2(128) W3(128) Wr1(128) Wr2(1)
    inp("fb", [H, 7], F32)        # cols: b2 b3 br1 recb npadneg (br2,gt0 row0)

    score_ap = nc.dram_tensor("score", [1, 1], F32, kind="ExternalOutput").ap()
    if _DEBUG:
        dbg_acc_ap = nc.dram_tensor("dbg_acc", [H, nt], F32,
                                    kind="ExternalOutput").ap()
        dbg_cvec_ap = nc.dram_tensor("dbg_cvec", [H, 1], F32,
                                     kind="ExternalOutput").ap()
        dbg_tot2_ap = nc.dram_tensor("dbg_tot2", [H, 1], F32,
                                     kind="ExternalOutput").ap()

    with tile.TileContext(nc) as tc:
        with ExitStack() as ctx:
            const = ctx.enter_context(tc.tile_pool(name="const", bufs=1))
            rbuf = ctx.enter_context(tc.tile_pool(name="rbuf", bufs=3))
            abuf = ctx.enter_context(tc.tile_pool(name="abuf", bufs=3))
            psZ1 = ctx.enter_context(tc.tile_pool(name="psZ1", bufs=2, space="PSUM"))
            psZ2 = ctx.enter_context(tc.tile_pool(name="psZ2", bufs=3, space="PSUM"))
            psZ3 = ctx.enter_context(tc.tile_pool(name="psZ3", bufs=3, space="PSUM"))

            wb = const.tile([H, HBLOB], FP16, tag="wb", name="wb")
            nc.sync.dma_start(out=wb, in_=d["wb"])
            fb = const.tile([H, 7], F32, tag="fb", name="fb")
            nc.sync.dma_start(out=fb, in_=d["fb"])
            W1z = wb[0:K1, 0:H]
            W2 = wb[:, H:2 * H]
            W3 = wb[:, 2 * H:3 * H]
            Wr1 = wb[:, 3 * H:4 * H]
            Wr2 = wb[:, 4 * H:4 * H + 1]
            b2 = fb[:, 0:1]
            b3 = fb[:, 1:2]
            br1 = fb[:, 2:3]
            recb = fb[:, 3:4]
            npadneg = fb[:, 4:5]
            br2 = fb[0:1, 5:6]
            gt0 = fb[0:1, 6:7]

            # warm the ACT table set before the pipeline needs relu
            warm = const.tile([1, 64], F32, tag="warm", name="warm")
            nc.vector.memset(warm[:, :], 0.0)
            nc.scalar.activation(out=warm[:, :], in_=warm[:, :], func=AF.Relu,
                                 bias=0.0, scale=1.0)

            acc = const.tile([H, nt], F32, tag="acc", name="acc")
            a3_last = const.tile([H, 512], F32, tag="a3_last", name="a3_last")
            zeros = const.tile([H, 512], F32, tag="zeros", name="zeros")
            nc.vector.memset(zeros[:, :], 0.0)

            nchunks = (nt + CHUNK - 1) // CHUNK
            PREFC = 2
            chunks, z1s, a1s, z2s, a2s, z3s = {}, {}, {}, {}, {}, {}

            def dma_chunk(c):
                lo = c * CHUNK * 512
                w = min(CHUNK * 512, nt * 512 - lo)
                rt = rbuf.tile([K1, CHUNK * 512], FP16, tag="rhs",
                               name=f"rhs{c}")
                nc.sync.dma_start(out=rt[:, 0:w], in_=d["rhsG"][:, lo:lo + w])
                chunks[c] = rt

            for c in range(min(PREFC, nchunks)):
                dma_chunk(c)

            # relu3 engine split: DVE for tail tiles (pad columns must round
            # identically to the extracted cvec) plus every 5th tile (~35%
            # total) to balance DVE vs ACT occupancy
            def r3_on_dve(tt):
                return (tt >= nt - 6) or (tt % 5 == 0)

            for step in range(nt + 6):
                t0 = step
                if t0 < nt:
                    c0, s0 = divmod(t0, CHUNK)
                    if s0 == 0 and c0 + PREFC < nchunks:
                        dma_chunk(c0 + PREFC)
                    z1 = psZ1.tile([H, 512], F32, tag="z1", name=f"z1_{t0}")
                    z1s[t0] = z1
                    nc.tensor.matmul(out=z1[:, :], lhsT=W1z,
                                     rhs=chunks[c0][:, 512 * s0:512 * (s0 + 1)],
                                     start=True, stop=True)
                t1 = step - 1
                if 0 <= t1 < nt:
                    a1 = abuf.tile([H, 512], FP16, tag="a1", name=f"a1_{t1}")
                    a1s[t1] = a1
                    nc.vector.tensor_scalar(out=a1[:, :],
                                            in0=z1s.pop(t1)[:, :],
                                            scalar1=0.0, scalar2=None,
                                            op0=ALU.max)
                t2 = step - 2
                if 0 <= t2 < nt:
                    z2 = psZ2.tile([H, 512], F32, tag="z2", name=f"z2_{t2}")
                    z2s[t2] = z2
                    nc.tensor.matmul(out=z2[:, :], lhsT=W2,
                                     rhs=a1s.pop(t2)[:, :],
                                     start=True, stop=True)
                t3 = step - 3
                if 0 <= t3 < nt:
                    a2 = abuf.tile([H, 512], FP16, tag="a2", name=f"a2_{t3}")
                    a2s[t3] = a2
                    nc.scalar.activation(out=a2[:, :], in_=z2s.pop(t3)[:, :],
                                         func=AF.Relu, bias=b2, scale=1.0)
                t4 = step - 4
                if 0 <= t4 < nt:
                    z3 = psZ3.tile([H, 512], F32, tag="z3", name=f"z3_{t4}")
                    z3s[t4] = z3
                    nc.tensor.matmul(out=z3[:, :], lhsT=W3,
                                     rhs=a2s.pop(t4)[:, :],
                                     start=True, stop=True)
                t5 = step - 5
                if 0 <= t5 < nt:
                    if t5 == nt - 1:
                        a3 = a3_last
                    else:
                        a3 = abuf.tile([H, 512], FP16, tag="a3",
                                       name=f"a3_{t5}")
                    z3ap = z3s.pop(t5)
                    if r3_on_dve(t5):
                        # out = max(z3 + b3, 0); accum = sum(out)
                        nc.vector.scalar_tensor_tensor(
                            out=a3[:, :], in0=z3ap[:, :],
                            scalar=b3, in1=zeros[:, :],
                            op0=ALU.add, op1=ALU.max,
                            accum_out=acc[:, t5:t5 + 1])
                    else:
                        nc.scalar.activation(out=a3[:, :], in_=z3ap[:, :],
                                             func=AF.Relu, bias=b3,
                                             scale=1.0,
                                             accum_out=acc[:, t5:t5 + 1])

            # ---- head ----
            if _DEBUG:
                nc.sync.dma_start(out=dbg_acc_ap, in_=acc[:, :])
                nc.sync.dma_start(out=dbg_cvec_ap, in_=a3_last[:, 511:512])
            tot = const.tile([H, 1], F32, tag="tot", name="tot")
            nc.vector.tensor_reduce(out=tot[:, :], in_=acc[:, :],
                                    axis=mybir.AxisListType.X, op=ALU.add)
            # tot2 = (cvec * -npad) + tot   removes pad-column contributions
            tot2 = const.tile([H, 1], F32, tag="tot2", name="tot2")
            nc.vector.scalar_tensor_tensor(out=tot2[:, :],
                                           in0=a3_last[:, 511:512],
                                           scalar=npadneg,
                                           in1=tot[:, :],
                                           op0=ALU.mult, op1=ALU.add)
            if _DEBUG:
                nc.sync.dma_start(out=dbg_tot2_ap, in_=tot2[:, :])
            repr_ = const.tile([H, 1], FP16, tag="repr", name="repr_")
            nc.vector.tensor_scalar(out=repr_[:, :], in0=tot2[:, :],
                                    scalar1=recb, scalar2=None,
                                    op0=ALU.mult)
            r1_ps = psZ2.tile([H, 1], F32, tag="z2", name="r1_ps")
            nc.tensor.matmul(out=r1_ps[:, :], lhsT=Wr1,
                             rhs=repr_[:, :], start=True, stop=True)
            r1 = const.tile([H, 1], FP16, tag="r1", name="r1")
            nc.scalar.activation(out=r1[:, :], in_=r1_ps[:, :], func=AF.Relu,
                                 bias=br1, scale=1.0)
            sc_ps = psZ3.tile([1, 1], F32, tag="z3", name="sc_ps")
            nc.tensor.matmul(out=sc_ps[:, :], lhsT=Wr2,
                             rhs=r1[:, :], start=True, stop=True)
            sc = const.tile([1, 1], F32, tag="sc", name="sc")
            nc.scalar.activation(out=sc[:, :], in_=sc_ps[:, :],
                                 func=AF.Identity, bias=br2,
                                 scale=1.0)
            scf = const.tile([1, 1], F32, tag="scf", name="scf")
            nc.vector.tensor_tensor(out=scf[:, :], in0=sc[:, :],
                                    in1=gt0, op=ALU.mult)
            nc.sync.dma_start(out=score_ap, in_=scf[:, :])

    nc.compile()
    return nc


def _get_nc(nt=NT_DEFAULT):
    if nt not in _CACHE:
        _CACHE[nt] = _build_nc(nt)
    return _CACHE[nt]


def kernel(protein_pos, ligand_pos, prot_emb, lig_emb,
           W1, b1, W2, b2, W3, b3, Wr1, br1, Wr2, br2,
           protein_atom_type, ligand_atom_type, protein_batch, ligand_batch):
    protein_pos = np.asarray(protein_pos, dtype=np.float32).reshape(B, P, 3)
    ligand_pos = np.asarray(ligand_pos, dtype=np.float32).reshape(B, L, 3)
    prot_emb = np.asarray(prot_emb, dtype=np.float32)
    lig_emb = np.asarray(lig_emb, dtype=np.float32)
    W1 = np.asarray(W1, dtype=np.float32)
    b1 = np.asarray(b1, dtype=np.float32)
    ptype = np.asarray(protein_atom_type).reshape(B, P)
    ltype = np.asarray(ligand_atom_type).reshape(B, L)

    # fused stage-1 weight: z1 = W1z.T @ [rb; onehot_p; onehot_l]
    PA = prot_emb @ W1[0:H, :]                      # [20, 128]
    LA = lig_emb @ W1[H:2 * H, :] + b1[None, :]     # [16, 128]
    W1z = np.concatenate([W1[2 * H:2 * H + RB, :], PA, LA], axis=0)

    wb = np.zeros((H, HBLOB), dtype=np.float16)
    wb[0:K1, 0:H] = W1z
    wb[:, H:2 * H] = np.asarray(W2, np.float32)
    wb[:, 2 * H:3 * H] = np.asarray(W3, np.float32)
    wb[:, 3 * H:4 * H] = np.asarray(Wr1, np.float32)
    wb[:, 4 * H] = np.asarray(Wr2, np.float32).reshape(H)

    centers = np.linspace(0.0, CUTOFF, RB, dtype=np.float32)

    # per-complex valid-pair extraction
    pis, lis, dists, cnts = [], [], [], []
    kmax = 0
    for b in range(B):
        diff = protein_pos[b][:, None, :] - ligand_pos[b][None, :, :]
        dist = np.sqrt((diff * diff).sum(-1, dtype=np.float32))
        pi, li = np.nonzero(dist < np.float32(CUTOFF))
        pis.append(pi); lis.append(li); dists.append(dist[pi, li])
        cnts.append(len(pi)); kmax = max(kmax, len(pi))

    nt = max(NT_DEFAULT, (kmax + 1 + 511) // 512)
    npair = nt * 512

    in_maps = []
    for b in range(B):
        pi, li, dv, cnt = pis[b], lis[b], dists[b], cnts[b]
        rhs = np.zeros((K1, npair), dtype=np.float32)
        rhs[0:RB, :cnt] = np.exp(
            -0.5 * ((dv[:, None] - centers[None, :]) / WIDTH) ** 2).T
        rhs[RB + ptype[b][pi], np.arange(cnt)] = 1.0
        rhs[RB + NPT + ltype[b][li], np.arange(cnt)] = 1.0
        fbb = np.zeros((H, 7), dtype=np.float32)
        fbb[:, 0] = np.asarray(b2, np.float32).reshape(H)
        fbb[:, 1] = np.asarray(b3, np.float32).reshape(H)
        fbb[:, 2] = np.asarray(br1, np.float32).reshape(H)
        fbb[:, 3] = 1.0 / max(cnt, 1.0)
        fbb[:, 4] = -(npair - cnt)
        fbb[0, 5] = np.asarray(br2, np.float32).reshape(())
        fbb[0, 6] = 1.0 if cnt > 0 else 0.0
        in_maps.append({"rhsG": rhs.astype(np.float16), "wb": wb, "fb": fbb})

    nc = _get_nc(nt)
    res = bass_utils.run_bass_kernel_spmd(nc, in_maps,
                                          core_ids=list(range(N_CORES)))
    out = np.array([res.results[b]["score"][0, 0] for b in range(B)],
                   dtype=np.float32)
    return out
